# revision 2
# baseline (speedup 1.0000x reference)
"""DilatedAttention Trainium2 kernel.

Reference semantics (hardcoded):
  b=4, S=8192, h=16, d=64; groups i=0,1,2 with
  seg=[2048,4096,8192], rate=[1,2,4], gsize=[6,5,5], hmin=[0,5,10], off=[0,1,2].
  Per group: positions idx = off::rate (globally), per-position attention over
  the group's g heads (scores g x g over d), softmax over k-head axis,
  x = attn @ V, then x /= x.sum(over all positions of the group per (b,head,d)),
  scatter-add into out, finally out /= 3.

Strategy: host gathers each (batch, group) into dense [npos, 6, 64] tiles
(5-head groups zero-padded + additive -1e30 key mask), distributes
(b, group0) -> core 2b is wrong; actually cores 0-3 = (b, g0) [64 tiles],
cores 4-7 = (b, g1)+(b, g2) [32+16 tiles + 16 dummy] so the SPMD program is a
uniform 64-tile loop. Per-unit position-sum normalization is done on device
with data-driven unit-indicator matmuls; host scatter-adds shards into the
full output (head-5 overlap between g0 and g1 sums correctly) and divides by 3.
"""

import sys

sys.path.insert(0, "/opt/trn_rl_repo")

import numpy as np

B, S, H, D = 4, 8192, 16, 64
NG = 3
SEG = [2048, 4096, 8192]
RATE = [1, 2, 4]
GS = [6, 5, 5]
HMIN = [0, 5, 10]
OFF = [0, 1, 2]
GP = 6  # padded head-group size
P = 128  # positions per tile
TPC = 64  # tiles per core
NU = 3  # max units per core
SCALE = 1.0 / 8.0  # 1/sqrt(64)
NEG = -1.0e30

_CACHED_NC = None


def _build_nc(tpc=TPC, debug=False):
    import concourse.bass as bass
    import concourse.bacc as bacc
    import concourse.tile as tile
    from concourse import mybir

    f32 = mybir.dt.float32
    nc = bacc.Bacc()

    q_d = nc.dram_tensor("q", [tpc * P, GP * D], f32, kind="ExternalInput")
    k_d = nc.dram_tensor("k", [tpc * P, GP * D], f32, kind="ExternalInput")
    v_d = nc.dram_tensor("v", [tpc * P, GP * D], f32, kind="ExternalInput")
    mb_d = nc.dram_tensor("mb", [tpc * P, GP * GP], f32, kind="ExternalInput")
    indp_d = nc.dram_tensor("indp", [P, tpc * NU], f32, kind="ExternalInput")
    indT_d = nc.dram_tensor("indT", [NU, tpc], f32, kind="ExternalInput")
    zfix_d = nc.dram_tensor("zfix", [NU, GP * D], f32, kind="ExternalInput")
    o_d = nc.dram_tensor("o", [tpc * P, GP * D], f32, kind="ExternalOutput")
    rzts_d = nc.dram_tensor("rzts", [tpc, GP * D], f32)
    xdbg_d = (
        nc.dram_tensor("xdbg", [tpc * P, GP * D], f32, kind="ExternalOutput")
        if debug
        else None
    )

    with tile.TileContext(nc) as tc:
        with (
            tc.tile_pool(name="io", bufs=3) as io,
            tc.tile_pool(name="work", bufs=2) as work,
            tc.tile_pool(name="small", bufs=3) as small,
            tc.tile_pool(name="xs", bufs=1) as xs,
            tc.tile_pool(name="singles", bufs=1) as singles,
            tc.tile_pool(name="psum", bufs=1, space="PSUM") as psum,
            tc.tile_pool(name="opool", bufs=3) as opool,
        ):
            indp_sb = singles.tile([P, tpc * NU], f32)
            nc.sync.dma_start(out=indp_sb, in_=indp_d[:, :])
            indT_sb = singles.tile([NU, tpc], f32)
            nc.sync.dma_start(out=indT_sb, in_=indT_d[:, :])
            zfix_sb = singles.tile([NU, GP * D], f32)
            nc.sync.dma_start(out=zfix_sb, in_=zfix_d[:, :])

            xall = xs.tile([P, tpc, GP * D], f32)
            zu_ps = psum.tile([NU, GP * D], f32)

            for t in range(tpc):
                r0 = t * P
                q_sb = io.tile([P, GP, D], f32, tag="q")
                k_sb = io.tile([P, GP, D], f32, tag="k")
                v_sb = io.tile([P, GP, D], f32, tag="v")
                mb_sb = io.tile([P, GP, GP], f32, tag="mb")
                nc.sync.dma_start(out=q_sb, in_=q_d[r0 : r0 + P, :])
                nc.sync.dma_start(out=k_sb, in_=k_d[r0 : r0 + P, :])
                nc.sync.dma_start(out=v_sb, in_=v_d[r0 : r0 + P, :])
                nc.sync.dma_start(out=mb_sb, in_=mb_d[r0 : r0 + P, :])

                # prod1[p, (q k), d] = (Q[p,q,d] * SCALE) * K[p,k,d]
                prod1 = work.tile([P, GP * GP, D], f32, tag="prod1")
                for qh in range(GP):
                    q_ap = (
                        q_sb[:, qh, :].unsqueeze(1).broadcast_to([P, GP, D])
                    )
                    nc.vector.tensor_mul(
                        out=prod1[:, qh * GP : (qh + 1) * GP, :],
                        in0=q_ap,
                        in1=k_sb[:],
                    )

                # scores[p, (q k)] = sum_d prod1 + maskbias
                scores = small.tile([P, GP * GP], f32, tag="scores")
                nc.vector.tensor_reduce(
                    out=scores[:],
                    in_=prod1[:],
                    axis=mybir.AxisListType.X,
                    op=mybir.AluOpType.add,
                )
                nc.vector.tensor_add(
                    out=scores[:], in0=scores[:], in1=mb_sb[:].rearrange("p q k -> p (q k)")
                )

                # e = exp(scores)
                e_sb = small.tile([P, GP, GP], f32, tag="e")
                nc.scalar.activation(
                    out=e_sb[:].rearrange("p q k -> p (q k)"),
                    in_=scores[:],
                    func=mybir.ActivationFunctionType.Exp,
                )

                # denom[p, q] = sum_k e ; rd = 1/denom
                den = small.tile([P, GP], f32, tag="den")
                nc.vector.tensor_reduce(
                    out=den[:],
                    in_=e_sb[:],
                    axis=mybir.AxisListType.X,
                    op=mybir.AluOpType.add,
                )
                rd = small.tile([P, GP], f32, tag="rd")
                nc.vector.reciprocal(out=rd[:], in_=den[:])

                # attn[p, q, k] = e * rd[q]
                attn = small.tile([P, GP, GP], f32, tag="attn")
                rd_ap = rd[:].unsqueeze(2).broadcast_to([P, GP, GP])
                nc.vector.tensor_mul(out=attn[:], in0=e_sb[:], in1=rd_ap)

                # prod2[p, (q d), k] = attn[p,q,k] * V[p,k,d]
                prod2 = work.tile([P, GP * D, GP], f32, tag="prod2")
                v_t = v_sb[:].transpose([0, 2, 1])  # [P, d, k]
                for qh in range(GP):
                    attn_ap = (
                        attn[:, qh, :].unsqueeze(1).broadcast_to([P, D, GP])
                    )
                    nc.vector.tensor_mul(
                        out=prod2[:, qh * D : (qh + 1) * D, :],
                        in0=attn_ap,
                        in1=v_t,
                    )

                # x[p, (q,d)] = sum_k prod2
                x_slice = xall[:, t, :]
                nc.vector.tensor_reduce(
                    out=x_slice,
                    in_=prod2[:],
                    axis=mybir.AxisListType.X,
                    op=mybir.AluOpType.add,
                )

                if debug:
                    nc.sync.dma_start(
                        out=xdbg_d[t * P : (t + 1) * P, :], in_=x_slice
                    )

                # accumulate per-unit position sums: zu[u,:] += ind[t,u]*x[p,:]
                nc.tensor.matmul(
                    zu_ps[:],
                    indp_sb[:, t * NU : (t + 1) * NU],
                    x_slice,
                    start=(t == 0),
                    stop=(t == tpc - 1),
                )

            # ---- epilogue: per-unit Z, reciprocal, per-tile RZ rows ----
            zu_sb = singles.tile([NU, GP * D], f32)
            nc.vector.tensor_add(out=zu_sb[:], in0=zu_ps[:], in1=zfix_sb[:])
            rz_sb = singles.tile([NU, GP * D], f32)
            nc.vector.reciprocal(out=rz_sb[:], in_=zu_sb[:])

            rzt_ps = psum.tile([tpc, GP * D], f32)
            nc.tensor.matmul(rzt_ps[:], indT_sb[:], rz_sb[:], start=True, stop=True)
            rzt_sb = singles.tile([tpc, GP * D], f32)
            nc.scalar.copy(out=rzt_sb[:], in_=rzt_ps[:])
            nc.sync.dma_start(out=rzts_d[:, :], in_=rzt_sb[:])

            # ---- pass 2: scale x tiles and write out ----
            for t in range(tpc):
                rzb = opool.tile([P, GP * D], f32, tag="rzb")
                rz_src = bass.AP(
                    tensor=rzts_d.tensor if hasattr(rzts_d, "tensor") else rzts_d,
                    offset=t * (GP * D),
                    ap=[[0, P], [1, GP * D]],
                )
                nc.sync.dma_start(out=rzb[:], in_=rz_src)
                ot = opool.tile([P, GP * D], f32, tag="ot")
                nc.vector.tensor_mul(out=ot[:], in0=xall[:, t, :], in1=rzb[:])
                nc.sync.dma_start(out=o_d[t * P : (t + 1) * P, :], in_=ot[:])

    nc.finalize()
    return nc


def _host_pack(query, key, value):
    """Build per-core input maps. Returns (in_maps, scatter_plan).

    scatter_plan: list per core of (tile_lo, tile_hi, b, gi) unit ranges.
    """
    in_maps = []
    plans = []
    for core in range(8):
        if core < 4:
            b = core
            units = [(0,)]
            unit_list = [(b, 0)]
        else:
            b = core - 4
            unit_list = [(b, 1), (b, 2)]
        qt = np.zeros((TPC * P, GP * D), dtype=np.float32)
        kt = np.zeros((TPC * P, GP * D), dtype=np.float32)
        vt = np.zeros((TPC * P, GP * D), dtype=np.float32)
        mbt = np.zeros((TPC * P, GP * GP), dtype=np.float32)
        ind = np.zeros((TPC, NU), dtype=np.float32)
        zfix = np.ones((NU, GP * D), dtype=np.float32)
        plan = []
        t0 = 0
        for u, (bb, gi) in enumerate(unit_list):
            g = GS[gi]
            idx = np.arange(OFF[gi], S, RATE[gi])
            npos = idx.shape[0]
            ntile = npos // P
            rows = slice(t0 * P, t0 * P + npos)
            qg = query[bb, idx, HMIN[gi] : HMIN[gi] + g, :]  # [npos, g, 64]
            kg = key[bb, idx, HMIN[gi] : HMIN[gi] + g, :]
            vg = value[bb, idx, HMIN[gi] : HMIN[gi] + g, :]
            qt[rows, : g * D] = qg.reshape(npos, g * D) * SCALE
            kt[rows, : g * D] = kg.reshape(npos, g * D)
            vt[rows, : g * D] = vg.reshape(npos, g * D)
            mb = np.zeros((GP, GP), dtype=np.float32)
            mb[:, g:] = NEG
            mbt[rows, :] = mb.reshape(1, GP * GP)
            ind[t0 : t0 + ntile, u] = 1.0
            zfix[u, :] = 0.0
            plan.append((t0, t0 + ntile, bb, gi))
            t0 += ntile
        # dummy tiles [t0, TPC): zeros everywhere; mb zeros -> denom=6 finite;
        # they fall in unit NU-1... give them no unit (ind rows zero) so they
        # don't touch real Z. Their RZ row = 0 -> out 0. Fine.
        in_maps.append(
            {
                "q": qt,
                "k": kt,
                "v": vt,
                "mb": mbt,
                "indp": np.tile(ind.reshape(1, TPC * NU), (P, 1)),
                "indT": np.ascontiguousarray(ind.T),
                "zfix": zfix,
            }
        )
        plans.append(plan)
    return in_maps, plans


LAST_EXEC_NS = None


def kernel(query, key, value):
    global _CACHED_NC, LAST_EXEC_NS
    query = np.asarray(query, dtype=np.float32)
    key = np.asarray(key, dtype=np.float32)
    value = np.asarray(value, dtype=np.float32)

    import os

    from concourse.bass_utils import run_bass_kernel_spmd

    if _CACHED_NC is None:
        _CACHED_NC = _build_nc()
    nc = _CACHED_NC

    in_maps, plans = _host_pack(query, key, value)
    kw = {}
    if os.environ.get("KERNEL_TRACE"):
        kw = dict(trace=True)
        tdir = os.environ.get("KERNEL_TRACE_DIR")
        if tdir:
            os.makedirs(tdir, exist_ok=True)
            kw["tmpdir"] = tdir
    try:
        res = run_bass_kernel_spmd(nc, in_maps, list(range(8)), **kw)
    except Exception:
        if not kw:
            raise
        res = run_bass_kernel_spmd(nc, in_maps, list(range(8)))
    if getattr(res, "exec_time_ns", None):
        LAST_EXEC_NS = res.exec_time_ns
    results = res.results

    out = np.zeros((B, S, H, D), dtype=np.float32)
    for core in range(8):
        o = np.asarray(results[core]["o"]).reshape(TPC * P, GP, D)
        for t0, t1, bb, gi in plans[core]:
            g = GS[gi]
            idx = np.arange(OFF[gi], S, RATE[gi])
            xn = o[t0 * P : t0 * P + idx.shape[0], :g, :]
            out[bb, idx, HMIN[gi] : HMIN[gi] + g, :] += xn
    out /= NG
    return out



# revision 7
# speedup vs baseline: 1.3506x; 1.3506x over previous
"""DilatedAttention Trainium2 kernel (v2).

Reference semantics (hardcoded):
  b=4, S=8192, h=16, d=64; groups i=0,1,2 with
  seg=[2048,4096,8192], rate=[1,2,4], gsize=[6,5,5], hmin=[0,5,10], off=[0,1,2].
  Per group: positions idx = off::rate (globally); per-position attention over
  the group's g heads (g x g scores over d=64), softmax over the k-head axis,
  x = attn @ V, then x /= sum_over_positions(x) per (unit, head, d), scatter-add
  into out, finally out /= 3.

v2 layout:
  - Tight packing per group size: g0 -> [pos, 6, 64] tiles, g1/g2 -> [pos, 5, 64]
    (no head padding, no masks).
  - 8 cores, SPMD-uniform: every core runs 32 g6-tiles + 24 g5-tiles of 128
    positions. Core pair (2b, 2b+1) owns batch b:
      core 2b   : g6 tiles 0..31 of (b,g0); g5 = 16 tiles (b,g2) + tiles 0..7 of (b,g1)
      core 2b+1 : g6 tiles 32..63 of (b,g0); g5 = tiles 8..31 of (b,g1)
  - Device computes xn = softmax(QK^T/8) @ V per position plus per-section
    partial position-sums Z (PE ones-matmul into PSUM). Host combines Z in
    fp64, folds 1/(3Z) into the scatter-add. No second device pass.
"""

import sys

sys.path.insert(0, "/opt/trn_rl_repo")

import numpy as np

B, S, H, D = 4, 8192, 16, 64
NG = 3
SEG = [2048, 4096, 8192]
RATE = [1, 2, 4]
GS = [6, 5, 5]
HMIN = [0, 5, 10]
OFF = [0, 1, 2]
P = 128  # positions per tile
N6 = 32  # g6 tiles per core
N5 = 24  # g5 tiles per core
SCALE = 1.0 / 8.0  # 1/sqrt(64)

_CACHED_NC = None


def _section(nc, tc, pools, qd, kd, vd, od, zrows, ntiles, g):
    """One attention section: ntiles tiles of [P, g, 64].

    zrows: list of (psum_tile, t_lo, t_hi) -> accumulate position-sums of
    tiles [t_lo, t_hi) into that [1, g*D] PSUM tile.
    """
    import concourse.bass as bass
    from concourse import mybir

    f32 = mybir.dt.float32
    io, work, small, singles = pools
    gd = g * D

    ones = singles.tile([P, 1], f32, tag=f"ones{g}")
    nc.vector.memset(ones[:], 1.0)

    for t in range(ntiles):
        r0 = t * P
        q_sb = io.tile([P, g, D], f32, tag=f"q{g}")
        k_sb = io.tile([P, g, D], f32, tag=f"k{g}")
        v_sb = io.tile([P, g, D], f32, tag=f"v{g}")
        nc.sync.dma_start(out=q_sb, in_=qd[r0 : r0 + P, :])
        nc.sync.dma_start(out=k_sb, in_=kd[r0 : r0 + P, :])
        nc.sync.dma_start(out=v_sb, in_=vd[r0 : r0 + P, :])

        # prod1[p, q, k, d] = Q[p,q,d] * K[p,k,d]   (Q pre-scaled by 1/8)
        prod1 = work.tile([P, g, g, D], f32, tag=f"prod1_{g}")
        nc.vector.tensor_mul(
            out=prod1[:],
            in0=q_sb[:].unsqueeze(2).broadcast_to([P, g, g, D]),
            in1=k_sb[:].unsqueeze(1).broadcast_to([P, g, g, D]),
        )

        # scores[p, q, k] = sum_d prod1
        scores = small.tile([P, g, g], f32, tag=f"scores{g}")
        nc.vector.tensor_reduce(
            out=scores[:],
            in_=prod1[:],
            axis=mybir.AxisListType.X,
            op=mybir.AluOpType.add,
        )

        # e = exp(scores) on ACT
        e_sb = small.tile([P, g, g], f32, tag=f"e{g}")
        nc.scalar.activation(
            out=e_sb[:].rearrange("p q k -> p (q k)"),
            in_=scores[:].rearrange("p q k -> p (q k)"),
            func=mybir.ActivationFunctionType.Exp,
        )

        # den[p, q] = sum_k e ; rd = 1/den ; attn = e * rd
        den = small.tile([P, g], f32, tag=f"den{g}")
        nc.vector.tensor_reduce(
            out=den[:],
            in_=e_sb[:],
            axis=mybir.AxisListType.X,
            op=mybir.AluOpType.add,
        )
        rd = small.tile([P, g], f32, tag=f"rd{g}")
        nc.vector.reciprocal(out=rd[:], in_=den[:])
        attn = small.tile([P, g, g], f32, tag=f"attn{g}")
        nc.vector.tensor_mul(
            out=attn[:],
            in0=e_sb[:],
            in1=rd[:].unsqueeze(2).broadcast_to([P, g, g]),
        )

        # prod2[p, q, d, k] = attn[p,q,k] * V[p,k,d]
        prod2 = work.tile([P, g, D, g], f32, tag=f"prod2_{g}")
        v_t = v_sb[:].transpose([0, 2, 1])  # [P, d, k]
        nc.vector.tensor_mul(
            out=prod2[:],
            in0=attn[:].unsqueeze(2).broadcast_to([P, g, D, g]),
            in1=v_t.unsqueeze(1).broadcast_to([P, g, D, g]),
        )

        # xn[p, (q d)] = sum_k prod2
        xn = io.tile([P, gd], f32, tag=f"xn{g}")
        nc.vector.tensor_reduce(
            out=xn[:],
            in_=prod2[:],
            axis=mybir.AxisListType.X,
            op=mybir.AluOpType.add,
        )

        # partial Z accumulation on PE: z_ps[0, :] += sum_p xn[p, :]
        for z_ps, t_lo, t_hi in zrows:
            if t_lo <= t < t_hi:
                nc.tensor.matmul(
                    z_ps[:],
                    ones[:],
                    xn[:],
                    start=(t == t_lo),
                    stop=(t == t_hi - 1),
                )

        nc.sync.dma_start(out=od[r0 : r0 + P, :], in_=xn[:])


def _build_nc():
    import concourse.bacc as bacc
    import concourse.tile as tile
    from concourse import mybir

    f32 = mybir.dt.float32
    nc = bacc.Bacc()

    q6 = nc.dram_tensor("q6", [N6 * P, 6 * D], f32, kind="ExternalInput")
    k6 = nc.dram_tensor("k6", [N6 * P, 6 * D], f32, kind="ExternalInput")
    v6 = nc.dram_tensor("v6", [N6 * P, 6 * D], f32, kind="ExternalInput")
    q5 = nc.dram_tensor("q5", [N5 * P, 5 * D], f32, kind="ExternalInput")
    k5 = nc.dram_tensor("k5", [N5 * P, 5 * D], f32, kind="ExternalInput")
    v5 = nc.dram_tensor("v5", [N5 * P, 5 * D], f32, kind="ExternalInput")
    o6 = nc.dram_tensor("o6", [N6 * P, 6 * D], f32, kind="ExternalOutput")
    o5 = nc.dram_tensor("o5", [N5 * P, 5 * D], f32, kind="ExternalOutput")
    z6 = nc.dram_tensor("z6", [1, 6 * D], f32, kind="ExternalOutput")
    z5 = nc.dram_tensor("z5", [2, 5 * D], f32, kind="ExternalOutput")

    with tile.TileContext(nc) as tc:
        with (
            tc.tile_pool(name="io", bufs=3) as io,
            tc.tile_pool(name="work", bufs=2) as work,
            tc.tile_pool(name="small", bufs=3) as small,
            tc.tile_pool(name="singles", bufs=1) as singles,
            tc.tile_pool(name="psum", bufs=1, space="PSUM") as psum,
        ):
            pools = (io, work, small, singles)
            zu6_ps = psum.tile([1, 6 * D], f32)
            zu5a_ps = psum.tile([1, 5 * D], f32)
            zu5b_ps = psum.tile([1, 5 * D], f32)

            _section(nc, tc, pools, q6, k6, v6, o6, [(zu6_ps, 0, N6)], N6, 6)
            _section(
                nc, tc, pools, q5, k5, v5, o5,
                [(zu5a_ps, 0, 16), (zu5b_ps, 16, N5)], N5, 5,
            )

            zu6_sb = singles.tile([1, 6 * D], f32)
            nc.scalar.copy(out=zu6_sb[:], in_=zu6_ps[:])
            nc.sync.dma_start(out=z6[:, :], in_=zu6_sb[:])
            zu5a_sb = singles.tile([1, 5 * D], f32)
            nc.scalar.copy(out=zu5a_sb[:], in_=zu5a_ps[:])
            nc.sync.dma_start(out=z5[0:1, :], in_=zu5a_sb[:])
            zu5b_sb = singles.tile([1, 5 * D], f32)
            nc.scalar.copy(out=zu5b_sb[:], in_=zu5b_ps[:])
            nc.sync.dma_start(out=z5[1:2, :], in_=zu5b_sb[:])

    nc.finalize()
    return nc


def _gather(x, b, gi):
    """[npos, g, 64] strided gather for unit (b, gi)."""
    idx = np.arange(OFF[gi], S, RATE[gi])
    return np.ascontiguousarray(x[b, idx, HMIN[gi] : HMIN[gi] + GS[gi], :])


def _host_pack(query, key, value):
    in_maps = []
    for core in range(8):
        b, role = core // 2, core % 2
        qg0 = _gather(query, b, 0) * SCALE
        kg0 = _gather(key, b, 0)
        vg0 = _gather(value, b, 0)
        qg1 = _gather(query, b, 1) * SCALE
        kg1 = _gather(key, b, 1)
        vg1 = _gather(value, b, 1)
        if role == 0:
            sl6 = slice(0, N6 * P)
            qg2 = _gather(query, b, 2) * SCALE
            kg2 = _gather(key, b, 2)
            vg2 = _gather(value, b, 2)
            q5v = np.concatenate([qg2, qg1[: 8 * P]])
            k5v = np.concatenate([kg2, kg1[: 8 * P]])
            v5v = np.concatenate([vg2, vg1[: 8 * P]])
        else:
            sl6 = slice(N6 * P, 2 * N6 * P)
            q5v = qg1[8 * P : 32 * P]
            k5v = kg1[8 * P : 32 * P]
            v5v = vg1[8 * P : 32 * P]
        in_maps.append(
            {
                "q6": qg0[sl6].reshape(N6 * P, 6 * D),
                "k6": kg0[sl6].reshape(N6 * P, 6 * D),
                "v6": vg0[sl6].reshape(N6 * P, 6 * D),
                "q5": np.ascontiguousarray(q5v).reshape(N5 * P, 5 * D),
                "k5": np.ascontiguousarray(k5v).reshape(N5 * P, 5 * D),
                "v5": np.ascontiguousarray(v5v).reshape(N5 * P, 5 * D),
            }
        )
    return in_maps


LAST_EXEC_NS = None


def kernel(query, key, value):
    global _CACHED_NC, LAST_EXEC_NS
    query = np.asarray(query, dtype=np.float32)
    key = np.asarray(key, dtype=np.float32)
    value = np.asarray(value, dtype=np.float32)

    import os

    from concourse.bass_utils import run_bass_kernel_spmd

    if _CACHED_NC is None:
        _CACHED_NC = _build_nc()
    nc = _CACHED_NC

    in_maps = _host_pack(query, key, value)
    kw = {}
    if os.environ.get("KERNEL_TRACE"):
        kw = dict(trace=True)
        tdir = os.environ.get("KERNEL_TRACE_DIR")
        if tdir:
            os.makedirs(tdir, exist_ok=True)
            kw["tmpdir"] = tdir
    try:
        res = run_bass_kernel_spmd(nc, in_maps, list(range(8)), **kw)
    except Exception:
        if not kw:
            raise
        kw = {}
        res = run_bass_kernel_spmd(nc, in_maps, list(range(8)))
    if getattr(res, "exec_time_ns", None):
        LAST_EXEC_NS = res.exec_time_ns
    results = res.results

    # ---- host: combine Z partials (fp64), normalize during scatter ----
    # Z[unit] accumulators
    Z = np.zeros((B, NG), dtype=object)
    for b in range(B):
        for gi in range(NG):
            Z[b, gi] = np.zeros((GS[gi], D), dtype=np.float64)
    xn6 = {}
    xn5 = {}
    for core in range(8):
        b, role = core // 2, core % 2
        r = results[core]
        xn6[core] = np.asarray(r["o6"]).reshape(N6 * P, 6, D)
        xn5[core] = np.asarray(r["o5"]).reshape(N5 * P, 5, D)
        z6p = np.asarray(r["z6"]).reshape(6, D).astype(np.float64)
        z5p = np.asarray(r["z5"]).reshape(2, 5, D).astype(np.float64)
        Z[b, 0] += z6p
        if role == 0:
            Z[b, 2] += z5p[0]
            Z[b, 1] += z5p[1]
        else:
            Z[b, 1] += z5p[0] + z5p[1]

    out = np.zeros((B, S, H, D), dtype=np.float32)
    for b in range(B):
        rz = [(1.0 / (NG * Z[b, gi])).astype(np.float32) for gi in range(NG)]
        a_core, b_core = 2 * b, 2 * b + 1
        # group 0: positions 0..4095 on core A, 4096..8191 on core B
        idx0 = np.arange(OFF[0], S, RATE[0])
        x0 = np.concatenate([xn6[a_core], xn6[b_core]])
        out[b, idx0, HMIN[0] : HMIN[0] + 6, :] += x0 * rz[0]
        # group 2: tiles 0..15 of core A's g5 section
        idx2 = np.arange(OFF[2], S, RATE[2])
        out[b, idx2, HMIN[2] : HMIN[2] + 5, :] += xn5[a_core][: 16 * P] * rz[2]
        # group 1: tiles 16..23 of core A (pos 0..1023), all of core B
        idx1 = np.arange(OFF[1], S, RATE[1])
        x1 = np.concatenate([xn5[a_core][16 * P :], xn5[b_core]])
        out[b, idx1, HMIN[1] : HMIN[1] + 5, :] += x1 * rz[1]
    return out


# revision 8
# speedup vs baseline: 1.4696x; 1.0881x over previous
"""DilatedAttention Trainium2 kernel (v2).

Reference semantics (hardcoded):
  b=4, S=8192, h=16, d=64; groups i=0,1,2 with
  seg=[2048,4096,8192], rate=[1,2,4], gsize=[6,5,5], hmin=[0,5,10], off=[0,1,2].
  Per group: positions idx = off::rate (globally); per-position attention over
  the group's g heads (g x g scores over d=64), softmax over the k-head axis,
  x = attn @ V, then x /= sum_over_positions(x) per (unit, head, d), scatter-add
  into out, finally out /= 3.

v2 layout:
  - Tight packing per group size: g0 -> [pos, 6, 64] tiles, g1/g2 -> [pos, 5, 64]
    (no head padding, no masks).
  - 8 cores, SPMD-uniform: every core runs 32 g6-tiles + 24 g5-tiles of 128
    positions. Core pair (2b, 2b+1) owns batch b:
      core 2b   : g6 tiles 0..31 of (b,g0); g5 = 16 tiles (b,g2) + tiles 0..7 of (b,g1)
      core 2b+1 : g6 tiles 32..63 of (b,g0); g5 = tiles 8..31 of (b,g1)
  - Device computes xn = softmax(QK^T/8) @ V per position plus per-section
    partial position-sums Z (PE ones-matmul into PSUM). Host combines Z in
    fp64, folds 1/(3Z) into the scatter-add. No second device pass.
"""

import sys

sys.path.insert(0, "/opt/trn_rl_repo")

import numpy as np

B, S, H, D = 4, 8192, 16, 64
NG = 3
SEG = [2048, 4096, 8192]
RATE = [1, 2, 4]
GS = [6, 5, 5]
HMIN = [0, 5, 10]
OFF = [0, 1, 2]
P = 128  # positions per tile
N6 = 32  # g6 tiles per core
N5 = 24  # g5 tiles per core
SCALE = 1.0 / 8.0  # 1/sqrt(64)

_CACHED_NC = None


def _section(nc, tc, pools, qd, kd, vd, od, zrows, ntiles, g):
    """One attention section: ntiles tiles of [P, g, 64].

    zrows: list of (psum_tile, t_lo, t_hi) -> accumulate position-sums of
    tiles [t_lo, t_hi) into that [1, g*D] PSUM tile.
    """
    import concourse.bass as bass
    from concourse import mybir

    f32 = mybir.dt.float32
    io, work, small, singles = pools
    gd = g * D

    ones = singles.tile([P, 1], f32, tag=f"ones{g}")
    nc.vector.memset(ones[:], 1.0)

    for t in range(ntiles):
        r0 = t * P
        q_sb = io.tile([P, g, D], f32, tag=f"q{g}")
        k_sb = io.tile([P, g, D], f32, tag=f"k{g}")
        v_sb = io.tile([P, g, D], f32, tag=f"v{g}")
        nc.sync.dma_start(out=q_sb, in_=qd[r0 : r0 + P, :])
        nc.sync.dma_start(out=k_sb, in_=kd[r0 : r0 + P, :])
        nc.sync.dma_start(out=v_sb, in_=vd[r0 : r0 + P, :])

        # prod1[p, q, k, d] = Q[p,q,d] * K[p,k,d]   (Q pre-scaled by 1/8)
        prod1 = work.tile([P, g, g, D], f32, tag=f"prod1_{g}")
        nc.vector.tensor_mul(
            out=prod1[:],
            in0=q_sb[:].unsqueeze(2).broadcast_to([P, g, g, D]),
            in1=k_sb[:].unsqueeze(1).broadcast_to([P, g, g, D]),
        )

        # scores[p, q, k] = sum_d prod1
        scores = small.tile([P, g, g], f32, tag=f"scores{g}")
        nc.vector.tensor_reduce(
            out=scores[:],
            in_=prod1[:],
            axis=mybir.AxisListType.X,
            op=mybir.AluOpType.add,
        )

        # e = exp(scores) on ACT
        e_sb = small.tile([P, g, g], f32, tag=f"e{g}")
        nc.scalar.activation(
            out=e_sb[:].rearrange("p q k -> p (q k)"),
            in_=scores[:].rearrange("p q k -> p (q k)"),
            func=mybir.ActivationFunctionType.Exp,
        )

        # den[p, q] = sum_k e ; rd = 1/den ; attn = e * rd
        den = small.tile([P, g], f32, tag=f"den{g}")
        nc.vector.tensor_reduce(
            out=den[:],
            in_=e_sb[:],
            axis=mybir.AxisListType.X,
            op=mybir.AluOpType.add,
        )
        rd = small.tile([P, g], f32, tag=f"rd{g}")
        nc.vector.reciprocal(out=rd[:], in_=den[:])
        attn = small.tile([P, g, g], f32, tag=f"attn{g}")
        nc.vector.tensor_mul(
            out=attn[:],
            in0=e_sb[:],
            in1=rd[:].unsqueeze(2).broadcast_to([P, g, g]),
        )

        # prod2[p, q, k, d] = attn[p,q,k] * V[p,k,d]  (64-contiguous innermost)
        import os

        av_eng = nc.gpsimd if os.environ.get("AV_ENG", "vector") == "gpsimd" else nc.vector
        prod2 = work.tile([P, g, g, D], f32, tag=f"prod2_{g}")
        av_eng.tensor_mul(
            out=prod2[:],
            in0=attn[:].unsqueeze(3).broadcast_to([P, g, g, D]),
            in1=v_sb[:].unsqueeze(1).broadcast_to([P, g, g, D]),
        )

        # xn[p, q, d] = sum_k prod2 — pairwise tree keeps 64-contig segments
        xn = io.tile([P, g, D], f32, tag=f"xn{g}")
        if g == 6:
            h1 = work.tile([P, g, 3, D], f32, tag="h1_6")
            nc.vector.tensor_add(
                out=h1[:], in0=prod2[:, :, 0:3, :], in1=prod2[:, :, 3:6, :]
            )
            h2 = work.tile([P, g, 1, D], f32, tag="h2_6")
            nc.vector.tensor_add(
                out=h2[:], in0=h1[:, :, 0:1, :], in1=h1[:, :, 1:2, :]
            )
            nc.vector.tensor_add(
                out=xn[:].unsqueeze(2), in0=h2[:], in1=h1[:, :, 2:3, :]
            )
        else:
            h1 = work.tile([P, g, 2, D], f32, tag="h1_5")
            nc.vector.tensor_add(
                out=h1[:], in0=prod2[:, :, 0:2, :], in1=prod2[:, :, 2:4, :]
            )
            h2 = work.tile([P, g, 1, D], f32, tag="h2_5")
            nc.vector.tensor_add(
                out=h2[:], in0=h1[:, :, 0:1, :], in1=h1[:, :, 1:2, :]
            )
            nc.vector.tensor_add(
                out=xn[:].unsqueeze(2), in0=h2[:], in1=prod2[:, :, 4:5, :]
            )

        # partial Z accumulation on PE: z_ps[0, :] += sum_p xn[p, :]
        for z_ps, t_lo, t_hi in zrows:
            if t_lo <= t < t_hi:
                nc.tensor.matmul(
                    z_ps[:],
                    ones[:],
                    xn[:],
                    start=(t == t_lo),
                    stop=(t == t_hi - 1),
                )

        nc.sync.dma_start(out=od[r0 : r0 + P, :], in_=xn[:])


def _build_nc():
    import concourse.bacc as bacc
    import concourse.tile as tile
    from concourse import mybir

    f32 = mybir.dt.float32
    nc = bacc.Bacc()

    q6 = nc.dram_tensor("q6", [N6 * P, 6 * D], f32, kind="ExternalInput")
    k6 = nc.dram_tensor("k6", [N6 * P, 6 * D], f32, kind="ExternalInput")
    v6 = nc.dram_tensor("v6", [N6 * P, 6 * D], f32, kind="ExternalInput")
    q5 = nc.dram_tensor("q5", [N5 * P, 5 * D], f32, kind="ExternalInput")
    k5 = nc.dram_tensor("k5", [N5 * P, 5 * D], f32, kind="ExternalInput")
    v5 = nc.dram_tensor("v5", [N5 * P, 5 * D], f32, kind="ExternalInput")
    o6 = nc.dram_tensor("o6", [N6 * P, 6 * D], f32, kind="ExternalOutput")
    o5 = nc.dram_tensor("o5", [N5 * P, 5 * D], f32, kind="ExternalOutput")
    z6 = nc.dram_tensor("z6", [1, 6 * D], f32, kind="ExternalOutput")
    z5 = nc.dram_tensor("z5", [2, 5 * D], f32, kind="ExternalOutput")

    with tile.TileContext(nc) as tc:
        with (
            tc.tile_pool(name="io", bufs=3) as io,
            tc.tile_pool(name="work", bufs=2) as work,
            tc.tile_pool(name="small", bufs=3) as small,
            tc.tile_pool(name="singles", bufs=1) as singles,
            tc.tile_pool(name="psum", bufs=1, space="PSUM") as psum,
        ):
            pools = (io, work, small, singles)
            zu6_ps = psum.tile([1, 6 * D], f32)
            zu5a_ps = psum.tile([1, 5 * D], f32)
            zu5b_ps = psum.tile([1, 5 * D], f32)

            _section(nc, tc, pools, q6, k6, v6, o6, [(zu6_ps, 0, N6)], N6, 6)
            _section(
                nc, tc, pools, q5, k5, v5, o5,
                [(zu5a_ps, 0, 16), (zu5b_ps, 16, N5)], N5, 5,
            )

            zu6_sb = singles.tile([1, 6 * D], f32)
            nc.scalar.copy(out=zu6_sb[:], in_=zu6_ps[:])
            nc.sync.dma_start(out=z6[:, :], in_=zu6_sb[:])
            zu5a_sb = singles.tile([1, 5 * D], f32)
            nc.scalar.copy(out=zu5a_sb[:], in_=zu5a_ps[:])
            nc.sync.dma_start(out=z5[0:1, :], in_=zu5a_sb[:])
            zu5b_sb = singles.tile([1, 5 * D], f32)
            nc.scalar.copy(out=zu5b_sb[:], in_=zu5b_ps[:])
            nc.sync.dma_start(out=z5[1:2, :], in_=zu5b_sb[:])

    nc.finalize()
    return nc


def _gather(x, b, gi):
    """[npos, g, 64] strided gather for unit (b, gi)."""
    idx = np.arange(OFF[gi], S, RATE[gi])
    return np.ascontiguousarray(x[b, idx, HMIN[gi] : HMIN[gi] + GS[gi], :])


def _host_pack(query, key, value):
    in_maps = []
    for core in range(8):
        b, role = core // 2, core % 2
        qg0 = _gather(query, b, 0) * SCALE
        kg0 = _gather(key, b, 0)
        vg0 = _gather(value, b, 0)
        qg1 = _gather(query, b, 1) * SCALE
        kg1 = _gather(key, b, 1)
        vg1 = _gather(value, b, 1)
        if role == 0:
            sl6 = slice(0, N6 * P)
            qg2 = _gather(query, b, 2) * SCALE
            kg2 = _gather(key, b, 2)
            vg2 = _gather(value, b, 2)
            q5v = np.concatenate([qg2, qg1[: 8 * P]])
            k5v = np.concatenate([kg2, kg1[: 8 * P]])
            v5v = np.concatenate([vg2, vg1[: 8 * P]])
        else:
            sl6 = slice(N6 * P, 2 * N6 * P)
            q5v = qg1[8 * P : 32 * P]
            k5v = kg1[8 * P : 32 * P]
            v5v = vg1[8 * P : 32 * P]
        in_maps.append(
            {
                "q6": qg0[sl6].reshape(N6 * P, 6 * D),
                "k6": kg0[sl6].reshape(N6 * P, 6 * D),
                "v6": vg0[sl6].reshape(N6 * P, 6 * D),
                "q5": np.ascontiguousarray(q5v).reshape(N5 * P, 5 * D),
                "k5": np.ascontiguousarray(k5v).reshape(N5 * P, 5 * D),
                "v5": np.ascontiguousarray(v5v).reshape(N5 * P, 5 * D),
            }
        )
    return in_maps


LAST_EXEC_NS = None


def kernel(query, key, value):
    global _CACHED_NC, LAST_EXEC_NS
    query = np.asarray(query, dtype=np.float32)
    key = np.asarray(key, dtype=np.float32)
    value = np.asarray(value, dtype=np.float32)

    import os

    from concourse.bass_utils import run_bass_kernel_spmd

    if _CACHED_NC is None:
        _CACHED_NC = _build_nc()
    nc = _CACHED_NC

    in_maps = _host_pack(query, key, value)
    kw = {}
    if os.environ.get("KERNEL_TRACE"):
        kw = dict(trace=True)
        tdir = os.environ.get("KERNEL_TRACE_DIR")
        if tdir:
            os.makedirs(tdir, exist_ok=True)
            kw["tmpdir"] = tdir
    try:
        res = run_bass_kernel_spmd(nc, in_maps, list(range(8)), **kw)
    except Exception:
        if not kw:
            raise
        kw = {}
        res = run_bass_kernel_spmd(nc, in_maps, list(range(8)))
    if getattr(res, "exec_time_ns", None):
        LAST_EXEC_NS = res.exec_time_ns
    results = res.results

    # ---- host: combine Z partials (fp64), normalize during scatter ----
    # Z[unit] accumulators
    Z = np.zeros((B, NG), dtype=object)
    for b in range(B):
        for gi in range(NG):
            Z[b, gi] = np.zeros((GS[gi], D), dtype=np.float64)
    xn6 = {}
    xn5 = {}
    for core in range(8):
        b, role = core // 2, core % 2
        r = results[core]
        xn6[core] = np.asarray(r["o6"]).reshape(N6 * P, 6, D)
        xn5[core] = np.asarray(r["o5"]).reshape(N5 * P, 5, D)
        z6p = np.asarray(r["z6"]).reshape(6, D).astype(np.float64)
        z5p = np.asarray(r["z5"]).reshape(2, 5, D).astype(np.float64)
        Z[b, 0] += z6p
        if role == 0:
            Z[b, 2] += z5p[0]
            Z[b, 1] += z5p[1]
        else:
            Z[b, 1] += z5p[0] + z5p[1]

    out = np.zeros((B, S, H, D), dtype=np.float32)
    for b in range(B):
        rz = [(1.0 / (NG * Z[b, gi])).astype(np.float32) for gi in range(NG)]
        a_core, b_core = 2 * b, 2 * b + 1
        # group 0: positions 0..4095 on core A, 4096..8191 on core B
        idx0 = np.arange(OFF[0], S, RATE[0])
        x0 = np.concatenate([xn6[a_core], xn6[b_core]])
        out[b, idx0, HMIN[0] : HMIN[0] + 6, :] += x0 * rz[0]
        # group 2: tiles 0..15 of core A's g5 section
        idx2 = np.arange(OFF[2], S, RATE[2])
        out[b, idx2, HMIN[2] : HMIN[2] + 5, :] += xn5[a_core][: 16 * P] * rz[2]
        # group 1: tiles 16..23 of core A (pos 0..1023), all of core B
        idx1 = np.arange(OFF[1], S, RATE[1])
        x1 = np.concatenate([xn5[a_core][16 * P :], xn5[b_core]])
        out[b, idx1, HMIN[1] : HMIN[1] + 5, :] += x1 * rz[1]
    return out


# revision 10
# speedup vs baseline: 1.7581x; 1.1963x over previous
"""DilatedAttention Trainium2 kernel (v2).

Reference semantics (hardcoded):
  b=4, S=8192, h=16, d=64; groups i=0,1,2 with
  seg=[2048,4096,8192], rate=[1,2,4], gsize=[6,5,5], hmin=[0,5,10], off=[0,1,2].
  Per group: positions idx = off::rate (globally); per-position attention over
  the group's g heads (g x g scores over d=64), softmax over the k-head axis,
  x = attn @ V, then x /= sum_over_positions(x) per (unit, head, d), scatter-add
  into out, finally out /= 3.

v2 layout:
  - Tight packing per group size: g0 -> [pos, 6, 64] tiles, g1/g2 -> [pos, 5, 64]
    (no head padding, no masks).
  - 8 cores, SPMD-uniform: every core runs 32 g6-tiles + 24 g5-tiles of 128
    positions. Core pair (2b, 2b+1) owns batch b:
      core 2b   : g6 tiles 0..31 of (b,g0); g5 = 16 tiles (b,g2) + tiles 0..7 of (b,g1)
      core 2b+1 : g6 tiles 32..63 of (b,g0); g5 = tiles 8..31 of (b,g1)
  - Device computes xn = softmax(QK^T/8) @ V per position plus per-section
    partial position-sums Z (PE ones-matmul into PSUM). Host combines Z in
    fp64, folds 1/(3Z) into the scatter-add. No second device pass.
"""

import sys

sys.path.insert(0, "/opt/trn_rl_repo")

import numpy as np

B, S, H, D = 4, 8192, 16, 64
NG = 3
SEG = [2048, 4096, 8192]
RATE = [1, 2, 4]
GS = [6, 5, 5]
HMIN = [0, 5, 10]
OFF = [0, 1, 2]
P = 128  # positions per tile
N6 = 32  # g6 tiles per core
N5 = 24  # g5 tiles per core
SCALE = 1.0 / 8.0  # 1/sqrt(64)

_CACHED_NC = None


def _section(nc, tc, pools, qd, kd, vd, od, zrows, ntiles, g):
    """One attention section: ntiles tiles of [P, g, 64].

    zrows: list of (psum_tile, t_lo, t_hi) -> accumulate position-sums of
    tiles [t_lo, t_hi) into that [1, g*D] PSUM tile.
    """
    import concourse.bass as bass
    from concourse import mybir

    f32 = mybir.dt.float32
    io, work, small, singles = pools
    gd = g * D

    ones = singles.tile([P, 1], f32, tag=f"ones{g}")
    nc.vector.memset(ones[:], 1.0)

    for t in range(ntiles):
        r0 = t * P
        q_sb = io.tile([P, g, D], f32, tag=f"q{g}")
        k_sb = io.tile([P, g, D], f32, tag=f"k{g}")
        v_sb = io.tile([P, g, D], f32, tag=f"v{g}")
        nc.sync.dma_start(out=q_sb, in_=qd[r0 : r0 + P, :])
        nc.sync.dma_start(out=k_sb, in_=kd[r0 : r0 + P, :])
        nc.sync.dma_start(out=v_sb, in_=vd[r0 : r0 + P, :])

        # prod1[p, q, k, d] = Q[p,q,d] * K[p,k,d]   (Q pre-scaled by 1/8)
        prod1 = work.tile([P, g, g, D], f32, tag=f"prod1_{g}")
        nc.vector.tensor_mul(
            out=prod1[:],
            in0=q_sb[:].unsqueeze(2).broadcast_to([P, g, g, D]),
            in1=k_sb[:].unsqueeze(1).broadcast_to([P, g, g, D]),
        )

        # scores[p, q, k] = sum_d prod1
        scores = small.tile([P, g, g], f32, tag=f"scores{g}")
        nc.vector.tensor_reduce(
            out=scores[:],
            in_=prod1[:],
            axis=mybir.AxisListType.X,
            op=mybir.AluOpType.add,
        )

        # e = exp(scores) on ACT
        e_sb = small.tile([P, g, g], f32, tag=f"e{g}")
        nc.scalar.activation(
            out=e_sb[:].rearrange("p q k -> p (q k)"),
            in_=scores[:].rearrange("p q k -> p (q k)"),
            func=mybir.ActivationFunctionType.Exp,
        )

        # den[p, q] = sum_k e ; rd = 1/den ; attn = e * rd
        den = small.tile([P, g], f32, tag=f"den{g}")
        nc.vector.tensor_reduce(
            out=den[:],
            in_=e_sb[:],
            axis=mybir.AxisListType.X,
            op=mybir.AluOpType.add,
        )
        rd = small.tile([P, g], f32, tag=f"rd{g}")
        nc.vector.reciprocal(out=rd[:], in_=den[:])
        attn = small.tile([P, g, g], f32, tag=f"attn{g}")
        nc.vector.tensor_mul(
            out=attn[:],
            in0=e_sb[:],
            in1=rd[:].unsqueeze(2).broadcast_to([P, g, g]),
        )

        # prod2[p, q, k, d] = attn[p,q,k] * V[p,k,d]  (64-contiguous innermost)
        import os

        av_eng = nc.gpsimd if os.environ.get("AV_ENG", "gpsimd") == "gpsimd" else nc.vector
        prod2 = work.tile([P, g, g, D], f32, tag=f"prod2_{g}")
        av_eng.tensor_mul(
            out=prod2[:],
            in0=attn[:].unsqueeze(3).broadcast_to([P, g, g, D]),
            in1=v_sb[:].unsqueeze(1).broadcast_to([P, g, g, D]),
        )

        # xn[p, q, d] = sum_k prod2 — pairwise tree keeps 64-contig segments
        xn = io.tile([P, g, D], f32, tag=f"xn{g}")
        if g == 6:
            h1 = work.tile([P, g, 3, D], f32, tag="h1_6")
            nc.vector.tensor_add(
                out=h1[:], in0=prod2[:, :, 0:3, :], in1=prod2[:, :, 3:6, :]
            )
            h2 = work.tile([P, g, 1, D], f32, tag="h2_6")
            nc.vector.tensor_add(
                out=h2[:], in0=h1[:, :, 0:1, :], in1=h1[:, :, 1:2, :]
            )
            nc.vector.tensor_add(
                out=xn[:].unsqueeze(2), in0=h2[:], in1=h1[:, :, 2:3, :]
            )
        else:
            h1 = work.tile([P, g, 2, D], f32, tag="h1_5")
            nc.vector.tensor_add(
                out=h1[:], in0=prod2[:, :, 0:2, :], in1=prod2[:, :, 2:4, :]
            )
            h2 = work.tile([P, g, 1, D], f32, tag="h2_5")
            nc.vector.tensor_add(
                out=h2[:], in0=h1[:, :, 0:1, :], in1=h1[:, :, 1:2, :]
            )
            nc.vector.tensor_add(
                out=xn[:].unsqueeze(2), in0=h2[:], in1=prod2[:, :, 4:5, :]
            )

        # partial Z accumulation on PE: z_ps[0, :] += sum_p xn[p, :]
        for z_ps, t_lo, t_hi in zrows:
            if t_lo <= t < t_hi:
                nc.tensor.matmul(
                    z_ps[:],
                    ones[:],
                    xn[:],
                    start=(t == t_lo),
                    stop=(t == t_hi - 1),
                )

        nc.sync.dma_start(out=od[r0 : r0 + P, :], in_=xn[:])


def _build_nc():
    import concourse.bacc as bacc
    import concourse.tile as tile
    from concourse import mybir

    f32 = mybir.dt.float32
    nc = bacc.Bacc()

    q6 = nc.dram_tensor("q6", [N6 * P, 6 * D], f32, kind="ExternalInput")
    k6 = nc.dram_tensor("k6", [N6 * P, 6 * D], f32, kind="ExternalInput")
    v6 = nc.dram_tensor("v6", [N6 * P, 6 * D], f32, kind="ExternalInput")
    q5 = nc.dram_tensor("q5", [N5 * P, 5 * D], f32, kind="ExternalInput")
    k5 = nc.dram_tensor("k5", [N5 * P, 5 * D], f32, kind="ExternalInput")
    v5 = nc.dram_tensor("v5", [N5 * P, 5 * D], f32, kind="ExternalInput")
    o6 = nc.dram_tensor("o6", [N6 * P, 6 * D], f32, kind="ExternalOutput")
    o5 = nc.dram_tensor("o5", [N5 * P, 5 * D], f32, kind="ExternalOutput")
    z6 = nc.dram_tensor("z6", [1, 6 * D], f32, kind="ExternalOutput")
    z5 = nc.dram_tensor("z5", [2, 5 * D], f32, kind="ExternalOutput")

    with tile.TileContext(nc) as tc:
        with (
            tc.tile_pool(name="io", bufs=4) as io,
            tc.tile_pool(name="work", bufs=3) as work,
            tc.tile_pool(name="small", bufs=4) as small,
            tc.tile_pool(name="singles", bufs=1) as singles,
            tc.tile_pool(name="psum", bufs=1, space="PSUM") as psum,
        ):
            pools = (io, work, small, singles)
            zu6_ps = psum.tile([1, 6 * D], f32)
            zu5a_ps = psum.tile([1, 5 * D], f32)
            zu5b_ps = psum.tile([1, 5 * D], f32)

            _section(nc, tc, pools, q6, k6, v6, o6, [(zu6_ps, 0, N6)], N6, 6)
            _section(
                nc, tc, pools, q5, k5, v5, o5,
                [(zu5a_ps, 0, 16), (zu5b_ps, 16, N5)], N5, 5,
            )

            zu6_sb = singles.tile([1, 6 * D], f32)
            nc.scalar.copy(out=zu6_sb[:], in_=zu6_ps[:])
            nc.sync.dma_start(out=z6[:, :], in_=zu6_sb[:])
            zu5a_sb = singles.tile([1, 5 * D], f32)
            nc.scalar.copy(out=zu5a_sb[:], in_=zu5a_ps[:])
            nc.sync.dma_start(out=z5[0:1, :], in_=zu5a_sb[:])
            zu5b_sb = singles.tile([1, 5 * D], f32)
            nc.scalar.copy(out=zu5b_sb[:], in_=zu5b_ps[:])
            nc.sync.dma_start(out=z5[1:2, :], in_=zu5b_sb[:])

    nc.finalize()
    return nc


def _gather(x, b, gi):
    """[npos, g, 64] strided gather for unit (b, gi)."""
    idx = np.arange(OFF[gi], S, RATE[gi])
    return np.ascontiguousarray(x[b, idx, HMIN[gi] : HMIN[gi] + GS[gi], :])


def _host_pack(query, key, value):
    in_maps = []
    for core in range(8):
        b, role = core // 2, core % 2
        qg0 = _gather(query, b, 0) * SCALE
        kg0 = _gather(key, b, 0)
        vg0 = _gather(value, b, 0)
        qg1 = _gather(query, b, 1) * SCALE
        kg1 = _gather(key, b, 1)
        vg1 = _gather(value, b, 1)
        if role == 0:
            sl6 = slice(0, N6 * P)
            qg2 = _gather(query, b, 2) * SCALE
            kg2 = _gather(key, b, 2)
            vg2 = _gather(value, b, 2)
            q5v = np.concatenate([qg2, qg1[: 8 * P]])
            k5v = np.concatenate([kg2, kg1[: 8 * P]])
            v5v = np.concatenate([vg2, vg1[: 8 * P]])
        else:
            sl6 = slice(N6 * P, 2 * N6 * P)
            q5v = qg1[8 * P : 32 * P]
            k5v = kg1[8 * P : 32 * P]
            v5v = vg1[8 * P : 32 * P]
        in_maps.append(
            {
                "q6": qg0[sl6].reshape(N6 * P, 6 * D),
                "k6": kg0[sl6].reshape(N6 * P, 6 * D),
                "v6": vg0[sl6].reshape(N6 * P, 6 * D),
                "q5": np.ascontiguousarray(q5v).reshape(N5 * P, 5 * D),
                "k5": np.ascontiguousarray(k5v).reshape(N5 * P, 5 * D),
                "v5": np.ascontiguousarray(v5v).reshape(N5 * P, 5 * D),
            }
        )
    return in_maps


LAST_EXEC_NS = None


def kernel(query, key, value):
    global _CACHED_NC, LAST_EXEC_NS
    query = np.asarray(query, dtype=np.float32)
    key = np.asarray(key, dtype=np.float32)
    value = np.asarray(value, dtype=np.float32)

    import os

    from concourse.bass_utils import run_bass_kernel_spmd

    if _CACHED_NC is None:
        _CACHED_NC = _build_nc()
    nc = _CACHED_NC

    in_maps = _host_pack(query, key, value)
    kw = {}
    if os.environ.get("KERNEL_TRACE"):
        kw = dict(trace=True)
        tdir = os.environ.get("KERNEL_TRACE_DIR")
        if tdir:
            os.makedirs(tdir, exist_ok=True)
            kw["tmpdir"] = tdir
    try:
        res = run_bass_kernel_spmd(nc, in_maps, list(range(8)), **kw)
    except Exception:
        if not kw:
            raise
        kw = {}
        res = run_bass_kernel_spmd(nc, in_maps, list(range(8)))
    if getattr(res, "exec_time_ns", None):
        LAST_EXEC_NS = res.exec_time_ns
    results = res.results

    # ---- host: combine Z partials (fp64), normalize during scatter ----
    # Z[unit] accumulators
    Z = np.zeros((B, NG), dtype=object)
    for b in range(B):
        for gi in range(NG):
            Z[b, gi] = np.zeros((GS[gi], D), dtype=np.float64)
    xn6 = {}
    xn5 = {}
    for core in range(8):
        b, role = core // 2, core % 2
        r = results[core]
        xn6[core] = np.asarray(r["o6"]).reshape(N6 * P, 6, D)
        xn5[core] = np.asarray(r["o5"]).reshape(N5 * P, 5, D)
        z6p = np.asarray(r["z6"]).reshape(6, D).astype(np.float64)
        z5p = np.asarray(r["z5"]).reshape(2, 5, D).astype(np.float64)
        Z[b, 0] += z6p
        if role == 0:
            Z[b, 2] += z5p[0]
            Z[b, 1] += z5p[1]
        else:
            Z[b, 1] += z5p[0] + z5p[1]

    out = np.zeros((B, S, H, D), dtype=np.float32)
    for b in range(B):
        rz = [(1.0 / (NG * Z[b, gi])).astype(np.float32) for gi in range(NG)]
        a_core, b_core = 2 * b, 2 * b + 1
        # group 0: positions 0..4095 on core A, 4096..8191 on core B
        idx0 = np.arange(OFF[0], S, RATE[0])
        x0 = np.concatenate([xn6[a_core], xn6[b_core]])
        out[b, idx0, HMIN[0] : HMIN[0] + 6, :] += x0 * rz[0]
        # group 2: tiles 0..15 of core A's g5 section
        idx2 = np.arange(OFF[2], S, RATE[2])
        out[b, idx2, HMIN[2] : HMIN[2] + 5, :] += xn5[a_core][: 16 * P] * rz[2]
        # group 1: tiles 16..23 of core A (pos 0..1023), all of core B
        idx1 = np.arange(OFF[1], S, RATE[1])
        x1 = np.concatenate([xn5[a_core][16 * P :], xn5[b_core]])
        out[b, idx1, HMIN[1] : HMIN[1] + 5, :] += x1 * rz[1]
    return out


# revision 11
# speedup vs baseline: 2.4322x; 1.3834x over previous
"""DilatedAttention Trainium2 kernel (v2b).

Same math as v2 but the score computation runs in a d-transposed layout so
the d-reduction moves to the TensorEngine:

  - Q,K tiles are DMA'd in layout [(half,d)=128, (q, p')] (host pre-transposes;
    same byte count). prod1_t[p128=(h,d), q, k, p'] = Q*K on DVE (innermost p'
    64-contiguous).
  - Per q-head c: PE matmul with a [128,2] block-ones stationary reduces over
    d partitions -> scores chunk [2, g*64] in its own PSUM bank (fp32).
  - ACT applies Exp straight out of PSUM into e_t [2, g, g*64] (SBUF), one
    SBUF->SBUF DMA reshapes to the standard [128 pos, q, k] layout.
  - Softmax denominator, attn, AV product (std layout, 64-contig innermost,
    pairwise tree k-reduction) as v2. AV product engine env-switchable
    (AV_ENG=gpsimd|vector).
  - No on-device normalization or Z: host sums xn in fp64 and folds 1/(3Z)
    into the scatter-add.
"""

import sys

sys.path.insert(0, "/opt/trn_rl_repo")

import numpy as np

B, S, H, D = 4, 8192, 16, 64
NG = 3
SEG = [2048, 4096, 8192]
RATE = [1, 2, 4]
GS = [6, 5, 5]
HMIN = [0, 5, 10]
OFF = [0, 1, 2]
P = 128  # positions per tile
N6 = 32  # g6 tiles per core
N5 = 24  # g5 tiles per core
SCALE = 1.0 / 8.0

_CACHED_NC = None


def _section(nc, pools, qd, kd, vd, od, sc_ps, onesbd, ntiles, g):
    import os

    from concourse import mybir

    f32 = mybir.dt.float32
    io, work, small, singles = pools
    gd = g * D
    av_eng = (
        nc.gpsimd if os.environ.get("AV_ENG", "gpsimd") == "gpsimd" else nc.vector
    )

    for t in range(ntiles):
        r0 = t * P
        qt_sb = io.tile([P, g, D], f32, tag=f"qt{g}")  # [(h d), (q p')]
        kt_sb = io.tile([P, g, D], f32, tag=f"kt{g}")
        v_sb = io.tile([P, g, D], f32, tag=f"v{g}")
        nc.sync.dma_start(out=qt_sb, in_=qd[r0 : r0 + P, :])
        nc.sync.dma_start(out=kt_sb, in_=kd[r0 : r0 + P, :])
        nc.sync.dma_start(out=v_sb, in_=vd[r0 : r0 + P, :])

        # prod1_t[(h d), q, k, p'] = Qt[(h d), q, p'] * Kt[(h d), k, p']
        prod1 = work.tile([P, g, g, D], f32, tag=f"prod1_{g}")
        nc.vector.tensor_mul(
            out=prod1[:],
            in0=qt_sb[:].unsqueeze(2).broadcast_to([P, g, g, D]),
            in1=kt_sb[:].unsqueeze(1).broadcast_to([P, g, g, D]),
        )

        # PE: scores chunk c = sum_d prod1[(h d), c, :, :] -> PSUM [2, (k p')]
        e_t = small.tile([2, 64, g, g], f32, tag=f"et{g}")  # [h, p', q, k]
        for c in range(g):
            nc.tensor.matmul(
                sc_ps[c][:, 0:gd],
                onesbd[:],
                prod1[:, c, :, :],
                start=True,
                stop=True,
            )
            # ACT: exp straight out of PSUM, transposing (k p') -> (p' k)
            nc.scalar.activation(
                out=e_t[:, :, c, :],
                in_=sc_ps[c][:, 0:gd].rearrange("h (k p) -> h p k", k=g),
                func=mybir.ActivationFunctionType.Exp,
            )

        # reshape [h, p', q, k] -> [pos=(h p'), q, k] via SBUF->SBUF DMA
        e_sb = small.tile([P, g, g], f32, tag=f"e{g}")
        for h in range(2):
            nc.sync.dma_start(
                out=e_sb[h * 64 : (h + 1) * 64, :, :],
                in_=e_t[h : h + 1].rearrange("h p q k -> h p (q k)"),
            )

        # den[p, q] = sum_k e ; rd = 1/den ; attn = e * rd
        den = small.tile([P, g], f32, tag=f"den{g}")
        nc.vector.tensor_reduce(
            out=den[:],
            in_=e_sb[:],
            axis=mybir.AxisListType.X,
            op=mybir.AluOpType.add,
        )
        rd = small.tile([P, g], f32, tag=f"rd{g}")
        nc.vector.reciprocal(out=rd[:], in_=den[:])
        attn = small.tile([P, g, g], f32, tag=f"attn{g}")
        nc.vector.tensor_mul(
            out=attn[:],
            in0=e_sb[:],
            in1=rd[:].unsqueeze(2).broadcast_to([P, g, g]),
        )

        # prod2[p, q, k, d] = attn[p,q,k] * V[p,k,d]
        prod2 = work.tile([P, g, g, D], f32, tag=f"prod2_{g}")
        av_eng.tensor_mul(
            out=prod2[:],
            in0=attn[:].unsqueeze(3).broadcast_to([P, g, g, D]),
            in1=v_sb[:].unsqueeze(1).broadcast_to([P, g, g, D]),
        )

        # xn[p, q, d] = sum_k prod2 (pairwise tree, 64-contig)
        xn = io.tile([P, g, D], f32, tag=f"xn{g}")
        if g == 6:
            h1 = work.tile([P, g, 3, D], f32, tag="h1_6")
            nc.vector.tensor_add(
                out=h1[:], in0=prod2[:, :, 0:3, :], in1=prod2[:, :, 3:6, :]
            )
            h2 = work.tile([P, g, 1, D], f32, tag="h2_6")
            nc.vector.tensor_add(
                out=h2[:], in0=h1[:, :, 0:1, :], in1=h1[:, :, 1:2, :]
            )
            nc.vector.tensor_add(
                out=xn[:].unsqueeze(2), in0=h2[:], in1=h1[:, :, 2:3, :]
            )
        else:
            h1 = work.tile([P, g, 2, D], f32, tag="h1_5")
            nc.vector.tensor_add(
                out=h1[:], in0=prod2[:, :, 0:2, :], in1=prod2[:, :, 2:4, :]
            )
            h2 = work.tile([P, g, 1, D], f32, tag="h2_5")
            nc.vector.tensor_add(
                out=h2[:], in0=h1[:, :, 0:1, :], in1=h1[:, :, 1:2, :]
            )
            nc.vector.tensor_add(
                out=xn[:].unsqueeze(2), in0=h2[:], in1=prod2[:, :, 4:5, :]
            )

        nc.sync.dma_start(out=od[r0 : r0 + P, :], in_=xn[:])


def _build_nc():
    import concourse.bacc as bacc
    import concourse.tile as tile
    from concourse import mybir

    f32 = mybir.dt.float32
    nc = bacc.Bacc()

    q6 = nc.dram_tensor("q6", [N6 * P, 6 * D], f32, kind="ExternalInput")
    k6 = nc.dram_tensor("k6", [N6 * P, 6 * D], f32, kind="ExternalInput")
    v6 = nc.dram_tensor("v6", [N6 * P, 6 * D], f32, kind="ExternalInput")
    q5 = nc.dram_tensor("q5", [N5 * P, 5 * D], f32, kind="ExternalInput")
    k5 = nc.dram_tensor("k5", [N5 * P, 5 * D], f32, kind="ExternalInput")
    v5 = nc.dram_tensor("v5", [N5 * P, 5 * D], f32, kind="ExternalInput")
    ob_d = nc.dram_tensor("onesbd", [P, 2], f32, kind="ExternalInput")
    o6 = nc.dram_tensor("o6", [N6 * P, 6 * D], f32, kind="ExternalOutput")
    o5 = nc.dram_tensor("o5", [N5 * P, 5 * D], f32, kind="ExternalOutput")

    with tile.TileContext(nc) as tc:
        with (
            tc.tile_pool(name="io", bufs=4) as io,
            tc.tile_pool(name="work", bufs=2) as work,
            tc.tile_pool(name="small", bufs=3) as small,
            tc.tile_pool(name="singles", bufs=1) as singles,
            tc.tile_pool(name="psum", bufs=1, space="PSUM") as psum,
        ):
            pools = (io, work, small, singles)
            onesbd = singles.tile([P, 2], f32)
            nc.sync.dma_start(out=onesbd, in_=ob_d[:, :])
            sc_ps = [
                psum.tile([2, 6 * D], f32, tag=f"sc{c}", name=f"sc{c}")
                for c in range(6)
            ]

            _section(nc, pools, q6, k6, v6, o6, sc_ps, onesbd, N6, 6)
            _section(nc, pools, q5, k5, v5, o5, sc_ps, onesbd, N5, 5)

    nc.finalize()
    return nc


def _gather(x, b, gi):
    idx = np.arange(OFF[gi], S, RATE[gi])
    return np.ascontiguousarray(x[b, idx, HMIN[gi] : HMIN[gi] + GS[gi], :])


def _transp(a, g):
    """[npos, g, 64] -> transposed tile layout [npos, g*64] with
    row = h*64+d, col = q*64+p' per 128-position tile."""
    nt = a.shape[0] // P
    # [t, h, p', q, d] -> [t, h, d, q, p']
    at = a.reshape(nt, 2, 64, g, D).transpose(0, 1, 4, 3, 2)
    return np.ascontiguousarray(at).reshape(nt * P, g * D)


def _host_pack(query, key, value):
    in_maps = []
    onesbd = np.zeros((P, 2), dtype=np.float32)
    onesbd[0:64, 0] = 1.0
    onesbd[64:128, 1] = 1.0
    for core in range(8):
        b, role = core // 2, core % 2
        qg0 = _gather(query, b, 0) * SCALE
        kg0 = _gather(key, b, 0)
        vg0 = _gather(value, b, 0)
        qg1 = _gather(query, b, 1) * SCALE
        kg1 = _gather(key, b, 1)
        vg1 = _gather(value, b, 1)
        if role == 0:
            sl6 = slice(0, N6 * P)
            qg2 = _gather(query, b, 2) * SCALE
            kg2 = _gather(key, b, 2)
            vg2 = _gather(value, b, 2)
            q5v = np.concatenate([qg2, qg1[: 8 * P]])
            k5v = np.concatenate([kg2, kg1[: 8 * P]])
            v5v = np.concatenate([vg2, vg1[: 8 * P]])
        else:
            sl6 = slice(N6 * P, 2 * N6 * P)
            q5v = qg1[8 * P : 32 * P]
            k5v = kg1[8 * P : 32 * P]
            v5v = vg1[8 * P : 32 * P]
        in_maps.append(
            {
                "q6": _transp(qg0[sl6], 6),
                "k6": _transp(kg0[sl6], 6),
                "v6": vg0[sl6].reshape(N6 * P, 6 * D),
                "q5": _transp(np.ascontiguousarray(q5v), 5),
                "k5": _transp(np.ascontiguousarray(k5v), 5),
                "v5": np.ascontiguousarray(v5v).reshape(N5 * P, 5 * D),
                "onesbd": onesbd,
            }
        )
    return in_maps


LAST_EXEC_NS = None


def kernel(query, key, value):
    global _CACHED_NC, LAST_EXEC_NS
    query = np.asarray(query, dtype=np.float32)
    key = np.asarray(key, dtype=np.float32)
    value = np.asarray(value, dtype=np.float32)

    import os

    from concourse.bass_utils import run_bass_kernel_spmd

    if _CACHED_NC is None:
        _CACHED_NC = _build_nc()
    nc = _CACHED_NC

    in_maps = _host_pack(query, key, value)
    kw = {}
    if os.environ.get("KERNEL_TRACE"):
        kw = dict(trace=True)
        tdir = os.environ.get("KERNEL_TRACE_DIR")
        if tdir:
            os.makedirs(tdir, exist_ok=True)
            kw["tmpdir"] = tdir
    try:
        res = run_bass_kernel_spmd(nc, in_maps, list(range(8)), **kw)
    except Exception:
        if not kw:
            raise
        kw = {}
        res = run_bass_kernel_spmd(nc, in_maps, list(range(8)))
    if getattr(res, "exec_time_ns", None):
        LAST_EXEC_NS = res.exec_time_ns
    results = res.results

    # ---- host: fp64 Z from device xn, fold 1/(3Z) into scatter-add ----
    xn6, xn5 = {}, {}
    Z = {}
    for b in range(B):
        for gi in range(NG):
            Z[b, gi] = np.zeros((GS[gi], D), dtype=np.float64)
    for core in range(8):
        b, role = core // 2, core % 2
        r = results[core]
        xn6[core] = np.asarray(r["o6"]).reshape(N6 * P, 6, D)
        xn5[core] = np.asarray(r["o5"]).reshape(N5 * P, 5, D)
        Z[b, 0] += np.sum(xn6[core], axis=0, dtype=np.float64)
        if role == 0:
            Z[b, 2] += np.sum(xn5[core][: 16 * P], axis=0, dtype=np.float64)
            Z[b, 1] += np.sum(xn5[core][16 * P :], axis=0, dtype=np.float64)
        else:
            Z[b, 1] += np.sum(xn5[core], axis=0, dtype=np.float64)

    out = np.zeros((B, S, H, D), dtype=np.float32)
    for b in range(B):
        rz = [(1.0 / (NG * Z[b, gi])).astype(np.float32) for gi in range(NG)]
        a_core, b_core = 2 * b, 2 * b + 1
        idx0 = np.arange(OFF[0], S, RATE[0])
        x0 = np.concatenate([xn6[a_core], xn6[b_core]])
        out[b, idx0, HMIN[0] : HMIN[0] + 6, :] += x0 * rz[0]
        idx2 = np.arange(OFF[2], S, RATE[2])
        out[b, idx2, HMIN[2] : HMIN[2] + 5, :] += xn5[a_core][: 16 * P] * rz[2]
        idx1 = np.arange(OFF[1], S, RATE[1])
        x1 = np.concatenate([xn5[a_core][16 * P :], xn5[b_core]])
        out[b, idx1, HMIN[1] : HMIN[1] + 5, :] += x1 * rz[1]
    return out
